# revision 2
# baseline (speedup 1.0000x reference)
"""Trainium2 Bass kernel for the per-batch attention block.

Reference math (per batch b, with C=E=512, H=W=32 -> N=1024, heads=8, d=64):
    qkv = w_in @ x_flat                      # [3E, N]
    S_h = q_h^T k_h * heads**-0.5            # [N, N] per head
    P_h = softmax(S_h, axis=-1)
    o_h = v_h @ P_h^T                        # [d, N]
    out = w_out @ concat(o_h) + b_out + x_flat

Mapping: data-parallel over batch across 8 NeuronCores (B=8, one batch
element per core). Inside a core everything is computed in a transposed
"S^T" layout so the TensorEngine contracts along its partition axis with
no on-chip transposes:
  - q,k produced channel-major ([ch, pos]) from fp16 inputs; v produced
    position-major (v^T = x^T @ w_v^T) with a trailing ones-column per
    head so the softmax denominator falls out of the o-matmul (row 64).
  - S^T = k_h^T q_h (64-row tiles), exp on ScalarE with a -10 shift
    (cancels in normalization; keeps P in fp16 range), o = (v^T)^T @ P^T
    accumulated over position chunks in PSUM.
  - normalize: dens + o evacuated to SBUF on DVE, reciprocal on DVE,
    1/den broadcast via DRAM bounce (stride-0 read), multiply on GpSimd
    writing the f32r projection input directly.
  - output projection m-outer in f32r with early evac+DMA overlap.
Host side: inputs stream as fp16 (halved DMA), output returns fp16 and
the +x residual, bias add, and f32 upcast happen on the host.
End-to-end ~6.6e-4 relative error.
"""

import sys

if "/opt/trn_rl_repo" not in sys.path:
    sys.path.insert(0, "/opt/trn_rl_repo")

from contextlib import ExitStack, nullcontext

import numpy as np

import concourse.bass as bass
import concourse.tile as tile
from concourse import bacc, mybir
from concourse.bass_utils import run_bass_kernel_spmd

F32 = mybir.dt.float32
F32R = mybir.dt.float32r
F16 = mybir.dt.float16
F8V = mybir.dt.float8e4
F8P = mybir.dt.float8e5
ESHIFT = -10.0  # exp bias shift: keeps P = exp(S*scale-10) within fp16 range
EXP = mybir.ActivationFunctionType.Exp

C = 512
N = 1024
E = 512
HEADS = 8
D = 64
NH = D + 1  # ones column + 64 v-channels per head
SCALE = float(HEADS) ** -0.5
P = 128
N_CORES = 8


def _build(n_cores=N_CORES, reps=1, loop_io=False, o_fp8=False, e_m_outer=True,
           merged_evac=False, stagger_b=False):
    nc = bacc.Bacc(
        "TRN2", target_bir_lowering=False, debug=False, num_devices=n_cores
    )
    x_d = nc.dram_tensor("x", [C, N], F16, kind="ExternalInput").ap()
    wqkT_d = nc.dram_tensor("wqkT", [C, 2 * E], F16, kind="ExternalInput").ap()
    wvT_d = nc.dram_tensor("wvT", [C, E], F16, kind="ExternalInput").ap()
    woutT_d = nc.dram_tensor("woutT", [E, C], F32R, kind="ExternalInput").ap()
    out_d = nc.dram_tensor("out", [C, N], F16, kind="ExternalOutput").ap()

    with tile.TileContext(nc) as tc, ExitStack() as ctx:
        consts = ctx.enter_context(tc.tile_pool(name="consts", bufs=1))
        qk_pool = ctx.enter_context(tc.tile_pool(name="qk", bufs=1))
        vt_pool = ctx.enter_context(tc.tile_pool(name="vt", bufs=1))
        osb_pool = ctx.enter_context(tc.tile_pool(name="osb", bufs=1))
        misc_pool = ctx.enter_context(tc.tile_pool(name="misc", bufs=2))

        # ---- constants (outside any rep loop) ---------------------------
        ones_col_f32 = consts.tile([P, HEADS], F32, tag="ones_col", name="ones_col")
        nc.vector.memset(ones_col_f32[:], 1.0)
        eshift_sb = consts.tile([P, 1], F32, tag="eshift", name="eshift_sb")
        nc.vector.memset(eshift_sb[:], ESHIFT)
        warm = consts.tile([P, 512], F16, tag="warm", name="warm")
        nc.vector.memset(warm[:], 0.0)

        rep_outer = (
            tc.For_i(0, reps, 1, hint_engines=(mybir.EngineType.PE,))
            if reps > 1 and loop_io
            else nullcontext()
        )
        rep_inner = (
            tc.For_i(0, reps, 1, hint_engines=(mybir.EngineType.PE,))
            if reps > 1 and not loop_io
            else nullcontext()
        )
        with (
            tc.tile_pool(name="ps", bufs=1, space="PSUM") as ps,
            tc.tile_pool(name="pt", bufs=3) as pt_pool,
            tc.tile_pool(name="norm", bufs=2) as norm_pool,
            tc.tile_pool(name="dram", bufs=2, space="DRAM") as dram_pool,
            rep_outer,
        ):
            # ---- input loads: first-use order, balanced across queues ---
            # sync queue: x (gates phase B) then wvT (gates phase C).
            # scalar queue: wqkT (gates phase B) then woutT + bias (phase E).
            xf = []
            wqkT = []
            wvT = []
            woutT = []
            for c in range(4):
                tx = consts.tile([P, N], F16, tag=f"xf{c}", name=f"xf{c}")
                nc.sync.dma_start(tx[:], x_d[c * P : (c + 1) * P, :])
                xf.append(tx)
                tw = consts.tile([P, 2 * E], F16, tag=f"wqkT{c}", name=f"wqkT{c}")
                nc.scalar.dma_start(tw[:], wqkT_d[c * P : (c + 1) * P, :])
                wqkT.append(tw)
            for c in range(4):
                t = consts.tile([P, E], F16, tag=f"wvT{c}", name=f"wvT{c}")
                nc.sync.dma_start(t[:], wvT_d[c * P : (c + 1) * P, :])
                wvT.append(t)
            for e in range(4):
                t = consts.tile([P, C], F32R, tag=f"woutT{e}", name=f"woutT{e}")
                nc.scalar.dma_start(t[:], woutT_d[e * P : (e + 1) * P, :])
                woutT.append(t)

            # ---- PE warm-up during the initial DMA window ---------------
            warm_ps = ps.tile([P, 512], F32, tag="s0", name="warm_ps")
            for w in range(6):
                nc.tensor.matmul(
                    warm_ps[:], warm[:, 0:P], warm[:], start=True, stop=True
                )

            qk_sb = [None] * 8
            vt_sb = [None] * 8
            osb = []
            for j in range(4):
                t = osb_pool.tile([P, N], F32R, tag=f"osb{j}", name=f"osb{j}")
                osb.append(t)

            def emit_B(m, tag=None):
                psum = ps.tile(
                    [P, N], F32, tag=tag or f"s{m % 2}", name=f"psB{m}"
                )
                for c in range(4):
                    for ih in range(2):
                        nc.tensor.matmul(
                            psum[:, ih * 512 : (ih + 1) * 512],
                            wqkT[c][:, m * P : (m + 1) * P],
                            xf[c][:, ih * 512 : (ih + 1) * 512],
                            start=(c == 0),
                            stop=(c == 3),
                        )
                t = qk_pool.tile([P, N], F16, tag=f"qk{m}", name=f"qk{m}")
                nc.vector.tensor_copy(t[:], psum[:])
                qk_sb[m] = t

            def emit_C(n):
                psum = ps.tile([P, E], F32, tag=f"s{n % 2}", name=f"psC{n}")
                for c in range(4):
                    nc.tensor.matmul(
                        psum[:],
                        xf[c][:, n * P : (n + 1) * P],
                        wvT[c][:],
                        start=(c == 0),
                        stop=(c == 3),
                    )
                # per-head layout: [v0..v63, ones] so the denominator row of
                # the o-matmul lands on PSUM partition 64 (engine partition
                # slices must start 32-aligned, so v-rows start at 0).
                # fp8 path: vt tiles hold position-plane pairs [P, 2, H*NH]
                # (plane r = a-chunk 2t+r) for DoubleRow o-matmuls.
                if o_fp8:
                    tt = n // 2
                    r = n % 2
                    if r == 0:
                        vt_sb[tt] = vt_pool.tile(
                            [P, 2 * HEADS * NH], F8V, tag=f"vt{tt}", name=f"vt{tt}"
                        )
                    t3 = vt_sb[tt][:, r * HEADS * NH : (r + 1) * HEADS * NH].rearrange(
                        "p (h d) -> p h d", h=HEADS
                    )
                else:
                    t = vt_pool.tile(
                        [P, HEADS * NH], F16, tag=f"vt{n}", name=f"vt{n}"
                    )
                    t3 = t[:].rearrange("p (h d) -> p h d", h=HEADS)
                    vt_sb[n] = t
                nc.vector.tensor_copy(
                    t3[:, :, 0:D], psum[:].rearrange("p (h d) -> p h d", h=HEADS)
                )
                nc.vector.tensor_copy(
                    t3[:, :, D : D + 1],
                    ones_col_f32[:].rearrange("p (h o) -> p h o", o=1),
                )

            def emit_pair(j, mid_work=None):
                qt = qk_sb[j]
                kt = qk_sb[4 + j]
                o_ps0 = ps.tile([NH, N], F32, tag="o0", name=f"o_ps0_{j}")
                o_ps1 = ps.tile([NH, N], F32, tag="o1", name=f"o_ps1_{j}")
                for a in range(8):
                    if mid_work is not None:
                        mid_work(a)
                    s_ps0 = ps.tile([P, N], F32, tag="s0", name=f"s_ps0_{j}_{a}")
                    s_ps1 = ps.tile([P, N], F32, tag="s1", name=f"s_ps1_{j}_{a}")
                    for ih in range(2):
                        sl = slice(ih * 512, (ih + 1) * 512)
                        nc.tensor.matmul(
                            s_ps0[:, sl],
                            kt[0:64, a * P : (a + 1) * P],
                            qt[0:64, sl],
                            start=True,
                            stop=True,
                        )
                        nc.tensor.matmul(
                            s_ps1[:, sl],
                            kt[64:128, a * P : (a + 1) * P],
                            qt[64:128, sl],
                            start=True,
                            stop=True,
                        )
                    if o_fp8:
                        tt = a // 2
                        r = a % 2
                        if r == 0:
                            pt0 = pt_pool.tile(
                                [P, 2 * N], F8P, tag="pt0", name=f"pt0_{j}_{tt}"
                            )
                            pt1 = pt_pool.tile(
                                [P, 2 * N], F8P, tag="pt1", name=f"pt1_{j}_{tt}"
                            )
                        nc.scalar.activation(
                            pt0[:, r * N : (r + 1) * N],
                            s_ps0[:],
                            EXP,
                            scale=SCALE,
                            bias=eshift_sb[:],
                        )
                        nc.scalar.activation(
                            pt1[:, r * N : (r + 1) * N],
                            s_ps1[:],
                            EXP,
                            scale=SCALE,
                            bias=eshift_sb[:],
                        )
                        if r == 1:
                            pt0_3 = pt0[:].rearrange("p (r2 n) -> p r2 n", r2=2)
                            pt1_3 = pt1[:].rearrange("p (r2 n) -> p r2 n", r2=2)
                            vt3 = vt_sb[tt][:].rearrange(
                                "p (r2 hd) -> p r2 hd", r2=2
                            )
                            for ih in range(2):
                                sl = slice(ih * 512, (ih + 1) * 512)
                                nc.tensor.matmul(
                                    o_ps0[:, sl],
                                    vt3[:, :, (2 * j) * NH : (2 * j + 1) * NH],
                                    pt0_3[:, :, sl],
                                    start=(tt == 0),
                                    stop=(tt == 3),
                                    perf_mode=mybir.MatmulPerfMode.DoubleRow,
                                )
                                nc.tensor.matmul(
                                    o_ps1[:, sl],
                                    vt3[:, :, (2 * j + 1) * NH : (2 * j + 2) * NH],
                                    pt1_3[:, :, sl],
                                    start=(tt == 0),
                                    stop=(tt == 3),
                                    perf_mode=mybir.MatmulPerfMode.DoubleRow,
                                )
                    else:
                        pt0 = pt_pool.tile(
                            [P, N], F16, tag="pt0", name=f"pt0_{j}_{a}"
                        )
                        pt1 = pt_pool.tile(
                            [P, N], F16, tag="pt1", name=f"pt1_{j}_{a}"
                        )
                        nc.scalar.activation(
                            pt0[:], s_ps0[:], EXP, scale=SCALE, bias=eshift_sb[:]
                        )
                        nc.scalar.activation(
                            pt1[:], s_ps1[:], EXP, scale=SCALE, bias=eshift_sb[:]
                        )
                        for ih in range(2):
                            sl = slice(ih * 512, (ih + 1) * 512)
                            nc.tensor.matmul(
                                o_ps0[:, sl],
                                vt_sb[a][:, (2 * j) * NH : (2 * j + 1) * NH],
                                pt0[:, sl],
                                start=(a == 0),
                                stop=(a == 7),
                            )
                            nc.tensor.matmul(
                                o_ps1[:, sl],
                                vt_sb[a][:, (2 * j + 1) * NH : (2 * j + 2) * NH],
                                pt1[:, sl],
                                start=(a == 0),
                                stop=(a == 7),
                            )

                # normalize: row 64 of o_ps holds the softmax denominator.
                # Evacuate PSUM fast on DVE (dens for both heads into one
                # [2,N] tile + unnormalized o into SBUF), then the slow
                # 1/den broadcast (DRAM bounce, stride-0 read) and the
                # multiply run on Pool, off the DVE/PSUM critical path.
                o_sb = []
                if merged_evac:
                    for slot, o_ps in ((0, o_ps0), (1, o_ps1)):
                        h = 2 * j + slot
                        t = norm_pool.tile(
                            [NH, N], F32, tag=f"osb_raw{slot}", name=f"o_raw{h}"
                        )
                        nc.vector.tensor_copy(t[:], o_ps[:])
                        o_sb.append(t)
                    r2 = norm_pool.tile([1, 2 * N], F32, tag="r2", name=f"r2_{j}")
                    for slot in (0, 1):
                        nc.vector.reciprocal_approx_fast(
                            r2[:, slot * N : (slot + 1) * N],
                            o_sb[slot][D : D + 1, :],
                        )
                else:
                    den2 = norm_pool.tile(
                        [1, 2 * N], F32, tag="den2", name=f"den2_{j}"
                    )
                    for slot, o_ps in ((0, o_ps0), (1, o_ps1)):
                        h = 2 * j + slot
                        nc.vector.tensor_copy(
                            den2[:, slot * N : (slot + 1) * N], o_ps[D : D + 1, :]
                        )
                        t = norm_pool.tile(
                            [D, N], F32, tag=f"osb_raw{slot}", name=f"o_raw{h}"
                        )
                        nc.vector.tensor_copy(t[:], o_ps[0:D, :])
                        o_sb.append(t)
                    r2 = norm_pool.tile([1, 2 * N], F32, tag="r2", name=f"r2_{j}")
                    nc.vector.reciprocal_approx_fast(r2[:], den2[:])
                den_dram = dram_pool.tile([1, 2 * N], F32, tag="den", name=f"den{j}")
                eng = (nc.sync, nc.scalar)[j % 2]
                eng.dma_start(den_dram[:], r2[:])
                for slot in (0, 1):
                    r_bc = norm_pool.tile(
                        [D, N], F32, tag=f"rbc{slot}", name=f"rbc{2 * j + slot}"
                    )
                    row = den_dram[:, slot * N : (slot + 1) * N]
                    bc_src = bass.AP(
                        tensor=row.tensor,
                        offset=row.offset,
                        ap=[[0, D]] + list(row.ap[1:]),
                    )
                    eng.dma_start(r_bc[:], bc_src)
                    base = slot * D
                    nc.gpsimd.tensor_mul(
                        osb[j][base : base + D, :], o_sb[slot][0:D, :], r_bc[:]
                    )

            def emit_compute():
                emit_B(0)
                emit_B(4)
                for n in range(8):
                    emit_C(n)
                for j in range(4):
                    if j < 3:

                        def mid(a, jn=j + 1):
                            if stagger_b:
                                if a == 3:
                                    emit_B(jn)
                                elif a == 5:
                                    emit_B(jn + 4)
                            elif a == 4:
                                emit_B(jn)
                                emit_B(jn + 4)

                        emit_pair(j, mid_work=mid)
                    else:
                        emit_pair(j)

                # ---- phase E: output projection (bias + residual on host) ---
                tags_e = ("s0", "s1", "o0", "o1")
                if e_m_outer:
                    for m in range(4):
                        psum = ps.tile([P, N], F32, tag=tags_e[m], name=f"psE{m}")
                        for e in range(4):
                            for ih in range(2):
                                sl = slice(ih * 512, (ih + 1) * 512)
                                nc.tensor.matmul(
                                    psum[:, sl],
                                    woutT[e][:, m * P : (m + 1) * P],
                                    osb[e][:, sl],
                                    start=(e == 0),
                                    stop=(e == 3),
                                )
                        out_sb = misc_pool.tile(
                            [P, N], F16, tag="outsb", name=f"out_sb{m}"
                        )
                        nc.vector.tensor_copy(out_sb[:], psum[:])
                        for ih in range(2):
                            sl = slice(ih * 512, (ih + 1) * 512)
                            eng = (nc.sync, nc.scalar, nc.scalar, nc.sync)[m]
                            eng.dma_start(
                                out_d[m * P : (m + 1) * P, sl], out_sb[:, sl]
                            )
                else:
                    psums = [
                        ps.tile([P, N], F32, tag=t_, name=f"psE{m}")
                        for m, t_ in enumerate(tags_e)
                    ]
                    for e in range(4):
                        for m in range(4):
                            for ih in range(2):
                                sl = slice(ih * 512, (ih + 1) * 512)
                                nc.tensor.matmul(
                                    psums[m][:, sl],
                                    woutT[e][:, m * P : (m + 1) * P],
                                    osb[e][:, sl],
                                    start=(e == 0),
                                    stop=(e == 3),
                                )
                    for m in range(4):
                        out_sb = misc_pool.tile(
                            [P, N], F16, tag="outsb", name=f"out_sb{m}"
                        )
                        nc.vector.tensor_copy(out_sb[:], psums[m][:])
                        for ih in range(2):
                            sl = slice(ih * 512, (ih + 1) * 512)
                            eng = (nc.sync, nc.scalar, nc.scalar, nc.sync)[m]
                            eng.dma_start(
                                out_d[m * P : (m + 1) * P, sl], out_sb[:, sl]
                            )

            with rep_inner:
                emit_compute()

    nc.compile()
    return nc


_CACHE = {}


def _get_nc(reps=1):
    key = reps
    if key not in _CACHE:
        _CACHE[key] = _build(reps=reps)
    return _CACHE[key]


def _in_maps(x, w_in, w_out, b_out):
    B = x.shape[0]
    xf = np.ascontiguousarray(x.reshape(B, C, N), dtype=np.float16)
    wqkT = np.ascontiguousarray(w_in[: 2 * E].T, dtype=np.float16)
    wvT = np.ascontiguousarray(w_in[2 * E :].T, dtype=np.float16)
    woutT = np.ascontiguousarray(w_out.T, dtype=np.float32)
    return [
        {"x": xf[b], "wqkT": wqkT, "wvT": wvT, "woutT": woutT}
        for b in range(B)
    ]


def kernel(x, w_in, w_out, b_out, heads):
    x = np.asarray(x)
    w_in = np.asarray(w_in)
    w_out = np.asarray(w_out)
    b_out = np.asarray(b_out)
    B = x.shape[0]
    assert int(heads) == HEADS, f"kernel compiled for heads=8, got {heads}"
    assert x.shape == (B, C, 32, 32) and B == N_CORES

    in_maps = _in_maps(x, w_in, w_out, b_out)
    nc = _get_nc()
    res = run_bass_kernel_spmd(nc, in_maps, core_ids=list(range(N_CORES)))
    out = np.stack([r["out"].astype(np.float32) for r in res.results])
    out = out.reshape(B, C, 32, 32) + x.astype(np.float32)
    out += b_out.astype(np.float32).reshape(1, C, 1, 1)
    return out.astype(x.dtype, copy=False)


# revision 3
# speedup vs baseline: 1.2930x; 1.2930x over previous
"""Trainium2 Bass kernel for the per-batch attention block.

Reference math (per batch b, with C=E=512, H=W=32 -> N=1024, heads=8, d=64):
    qkv = w_in @ x_flat                      # [3E, N]
    S_h = q_h^T k_h * heads**-0.5            # [N, N] per head
    P_h = softmax(S_h, axis=-1)
    o_h = v_h @ P_h^T                        # [d, N]
    out = w_out @ concat(o_h) + b_out + x_flat

Mapping: data-parallel over batch across 8 NeuronCores (B=8, one batch
element per core). Inside a core everything runs in a transposed "S^T"
layout so the TensorEngine contracts along its partition axis with no
on-chip transposes:
  - q,k produced channel-major from fp16 inputs; v produced
    position-major with a trailing ones-column per head so the softmax
    denominator falls out of the o-matmul (PSUM row 64).
  - S^T = k_h^T q_h (64-row tiles), exp on ScalarE with a -10 shift
    (cancels in normalization; keeps P in fp16 range), o = (v^T)^T @ P^T
    accumulated over position chunks in PSUM.
  - normalize: dens + o evacuated to SBUF on DVE (GpSimd cannot read
    PSUM; custom DVE ops need partition-0 inputs), reciprocal on DVE,
    1/den broadcast via DRAM bounce (stride-0 read), multiply on GpSimd
    writing the f32r projection input directly.
  - output projection m-outer in f32r with early evac+DMA overlap.
I/O: inputs stream as fp16 across both HWDGE queues with wqkT issued in
m-block-first order (the first attention pair unblocks as soon as x
lands); PE warm-up matmuls run during the DMA window; the device output
is fp16 and the +x residual, bias add, and f32 upcast happen on the
host.  End-to-end ~6.6e-4 relative error.
"""

import sys

if "/opt/trn_rl_repo" not in sys.path:
    sys.path.insert(0, "/opt/trn_rl_repo")

from contextlib import ExitStack, nullcontext

import numpy as np

import concourse.bass as bass
import concourse.tile as tile
from concourse import bacc, mybir
from concourse.bass_utils import run_bass_kernel_spmd

F32 = mybir.dt.float32
F32R = mybir.dt.float32r
F16 = mybir.dt.float16
F8V = mybir.dt.float8e4
F8P = mybir.dt.float8e5
ESHIFT = -10.0  # exp bias shift: keeps P = exp(S*scale-10) within fp16 range
EXP = mybir.ActivationFunctionType.Exp

C = 512
N = 1024
E = 512
HEADS = 8
D = 64
NH = D + 1  # ones column + 64 v-channels per head
SCALE = float(HEADS) ** -0.5
P = 128
N_CORES = 8


def _build(n_cores=N_CORES, reps=1, loop_io=False, o_fp8=False, e_m_outer=True,
           merged_evac=False, stagger_b=False, split_bc=False,
           pt_bufs=3, norm_bufs=2):
    nc = bacc.Bacc(
        "TRN2", target_bir_lowering=False, debug=False, num_devices=n_cores
    )
    x_d = nc.dram_tensor("x", [C, N], F16, kind="ExternalInput").ap()
    wqkT_d = nc.dram_tensor("wqkT", [C, 2 * E], F16, kind="ExternalInput").ap()
    wvT_d = nc.dram_tensor("wvT", [C, E], F16, kind="ExternalInput").ap()
    woutT_d = nc.dram_tensor("woutT", [E, C], F32R, kind="ExternalInput").ap()
    out_d = nc.dram_tensor("out", [C, N], F16, kind="ExternalOutput").ap()

    with tile.TileContext(nc) as tc, ExitStack() as ctx:
        consts = ctx.enter_context(tc.tile_pool(name="consts", bufs=1))
        qk_pool = ctx.enter_context(tc.tile_pool(name="qk", bufs=1))
        vt_pool = ctx.enter_context(tc.tile_pool(name="vt", bufs=1))
        osb_pool = ctx.enter_context(tc.tile_pool(name="osb", bufs=1))
        misc_pool = ctx.enter_context(tc.tile_pool(name="misc", bufs=2))

        # ---- constants (outside any rep loop) ---------------------------
        ones_col_f32 = consts.tile([P, HEADS], F32, tag="ones_col", name="ones_col")
        nc.vector.memset(ones_col_f32[:], 1.0)
        eshift_sb = consts.tile([P, 1], F32, tag="eshift", name="eshift_sb")
        nc.vector.memset(eshift_sb[:], ESHIFT)
        warm = consts.tile([P, 512], F16, tag="warm", name="warm")
        nc.vector.memset(warm[:], 0.0)

        rep_outer = (
            tc.For_i(0, reps, 1, hint_engines=(mybir.EngineType.PE,))
            if reps > 1 and loop_io
            else nullcontext()
        )
        rep_inner = (
            tc.For_i(0, reps, 1, hint_engines=(mybir.EngineType.PE,))
            if reps > 1 and not loop_io
            else nullcontext()
        )
        with (
            tc.tile_pool(name="ps", bufs=1, space="PSUM") as ps,
            tc.tile_pool(name="pt", bufs=pt_bufs) as pt_pool,
            tc.tile_pool(name="norm", bufs=norm_bufs) as norm_pool,
            tc.tile_pool(name="dram", bufs=2, space="DRAM") as dram_pool,
            rep_outer,
        ):
            # ---- input loads: first-use order, balanced across queues ---
            # sync queue: x (gates phase B) then wvT (gates phase C).
            # scalar queue: wqkT (gates phase B) then woutT + bias (phase E).
            xf = []
            wqkT = []
            wvT = []
            woutT = []
            for c in range(4):
                tx = consts.tile([P, N], F16, tag=f"xf{c}", name=f"xf{c}")
                nc.sync.dma_start(tx[:], x_d[c * P : (c + 1) * P, :])
                xf.append(tx)
                tw = consts.tile([P, 2 * E], F16, tag=f"wqkT{c}", name=f"wqkT{c}")
                wqkT.append(tw)
            for blk in (0, 2, 1, 3):
                cs = slice(blk * 256, (blk + 1) * 256)
                for c in range(4):
                    nc.scalar.dma_start(
                        wqkT[c][:, cs], wqkT_d[c * P : (c + 1) * P, cs]
                    )
            for c in range(4):
                t = consts.tile([P, E], F16, tag=f"wvT{c}", name=f"wvT{c}")
                nc.sync.dma_start(t[:], wvT_d[c * P : (c + 1) * P, :])
                wvT.append(t)
            for e in range(4):
                t = consts.tile([P, C], F32R, tag=f"woutT{e}", name=f"woutT{e}")
                nc.scalar.dma_start(t[:], woutT_d[e * P : (e + 1) * P, :])
                woutT.append(t)

            # ---- PE warm-up during the initial DMA window ---------------
            warm_ps = ps.tile([P, 512], F32, tag="s0", name="warm_ps")
            for w in range(6):
                nc.tensor.matmul(
                    warm_ps[:], warm[:, 0:P], warm[:], start=True, stop=True
                )

            qk_sb = [None] * 8
            vt_sb = [None] * 8
            osb = []
            for j in range(4):
                t = osb_pool.tile([P, N], F32R, tag=f"osb{j}", name=f"osb{j}")
                osb.append(t)

            def emit_B(m, tag=None):
                psum = ps.tile(
                    [P, N], F32, tag=tag or f"s{m % 2}", name=f"psB{m}"
                )
                for c in range(4):
                    for ih in range(2):
                        nc.tensor.matmul(
                            psum[:, ih * 512 : (ih + 1) * 512],
                            wqkT[c][:, m * P : (m + 1) * P],
                            xf[c][:, ih * 512 : (ih + 1) * 512],
                            start=(c == 0),
                            stop=(c == 3),
                        )
                t = qk_pool.tile([P, N], F16, tag=f"qk{m}", name=f"qk{m}")
                nc.vector.tensor_copy(t[:], psum[:])
                qk_sb[m] = t

            def emit_C(n):
                psum = ps.tile([P, E], F32, tag=f"s{n % 2}", name=f"psC{n}")
                for c in range(4):
                    nc.tensor.matmul(
                        psum[:],
                        xf[c][:, n * P : (n + 1) * P],
                        wvT[c][:],
                        start=(c == 0),
                        stop=(c == 3),
                    )
                # per-head layout: [v0..v63, ones] so the denominator row of
                # the o-matmul lands on PSUM partition 64 (engine partition
                # slices must start 32-aligned, so v-rows start at 0).
                # fp8 path: vt tiles hold position-plane pairs [P, 2, H*NH]
                # (plane r = a-chunk 2t+r) for DoubleRow o-matmuls.
                if o_fp8:
                    tt = n // 2
                    r = n % 2
                    if r == 0:
                        vt_sb[tt] = vt_pool.tile(
                            [P, 2 * HEADS * NH], F8V, tag=f"vt{tt}", name=f"vt{tt}"
                        )
                    t3 = vt_sb[tt][:, r * HEADS * NH : (r + 1) * HEADS * NH].rearrange(
                        "p (h d) -> p h d", h=HEADS
                    )
                else:
                    t = vt_pool.tile(
                        [P, HEADS * NH], F16, tag=f"vt{n}", name=f"vt{n}"
                    )
                    t3 = t[:].rearrange("p (h d) -> p h d", h=HEADS)
                    vt_sb[n] = t
                nc.vector.tensor_copy(
                    t3[:, :, 0:D], psum[:].rearrange("p (h d) -> p h d", h=HEADS)
                )
                nc.vector.tensor_copy(
                    t3[:, :, D : D + 1],
                    ones_col_f32[:].rearrange("p (h o) -> p h o", o=1),
                )

            def emit_pair(j, mid_work=None):
                qt = qk_sb[j]
                kt = qk_sb[4 + j]
                o_ps0 = ps.tile([NH, N], F32, tag="o0", name=f"o_ps0_{j}")
                o_ps1 = ps.tile([NH, N], F32, tag="o1", name=f"o_ps1_{j}")
                for a in range(8):
                    if mid_work is not None:
                        mid_work(a)
                    s_ps0 = ps.tile([P, N], F32, tag="s0", name=f"s_ps0_{j}_{a}")
                    s_ps1 = ps.tile([P, N], F32, tag="s1", name=f"s_ps1_{j}_{a}")
                    for ih in range(2):
                        sl = slice(ih * 512, (ih + 1) * 512)
                        nc.tensor.matmul(
                            s_ps0[:, sl],
                            kt[0:64, a * P : (a + 1) * P],
                            qt[0:64, sl],
                            start=True,
                            stop=True,
                        )
                        nc.tensor.matmul(
                            s_ps1[:, sl],
                            kt[64:128, a * P : (a + 1) * P],
                            qt[64:128, sl],
                            start=True,
                            stop=True,
                        )
                    if o_fp8:
                        tt = a // 2
                        r = a % 2
                        if r == 0:
                            pt0 = pt_pool.tile(
                                [P, 2 * N], F8P, tag="pt0", name=f"pt0_{j}_{tt}"
                            )
                            pt1 = pt_pool.tile(
                                [P, 2 * N], F8P, tag="pt1", name=f"pt1_{j}_{tt}"
                            )
                        nc.scalar.activation(
                            pt0[:, r * N : (r + 1) * N],
                            s_ps0[:],
                            EXP,
                            scale=SCALE,
                            bias=eshift_sb[:],
                        )
                        nc.scalar.activation(
                            pt1[:, r * N : (r + 1) * N],
                            s_ps1[:],
                            EXP,
                            scale=SCALE,
                            bias=eshift_sb[:],
                        )
                        if r == 1:
                            pt0_3 = pt0[:].rearrange("p (r2 n) -> p r2 n", r2=2)
                            pt1_3 = pt1[:].rearrange("p (r2 n) -> p r2 n", r2=2)
                            vt3 = vt_sb[tt][:].rearrange(
                                "p (r2 hd) -> p r2 hd", r2=2
                            )
                            for ih in range(2):
                                sl = slice(ih * 512, (ih + 1) * 512)
                                nc.tensor.matmul(
                                    o_ps0[:, sl],
                                    vt3[:, :, (2 * j) * NH : (2 * j + 1) * NH],
                                    pt0_3[:, :, sl],
                                    start=(tt == 0),
                                    stop=(tt == 3),
                                    perf_mode=mybir.MatmulPerfMode.DoubleRow,
                                )
                                nc.tensor.matmul(
                                    o_ps1[:, sl],
                                    vt3[:, :, (2 * j + 1) * NH : (2 * j + 2) * NH],
                                    pt1_3[:, :, sl],
                                    start=(tt == 0),
                                    stop=(tt == 3),
                                    perf_mode=mybir.MatmulPerfMode.DoubleRow,
                                )
                    else:
                        pt0 = pt_pool.tile(
                            [P, N], F16, tag="pt0", name=f"pt0_{j}_{a}"
                        )
                        pt1 = pt_pool.tile(
                            [P, N], F16, tag="pt1", name=f"pt1_{j}_{a}"
                        )
                        nc.scalar.activation(
                            pt0[:], s_ps0[:], EXP, scale=SCALE, bias=eshift_sb[:]
                        )
                        nc.scalar.activation(
                            pt1[:], s_ps1[:], EXP, scale=SCALE, bias=eshift_sb[:]
                        )
                        for ih in range(2):
                            sl = slice(ih * 512, (ih + 1) * 512)
                            nc.tensor.matmul(
                                o_ps0[:, sl],
                                vt_sb[a][:, (2 * j) * NH : (2 * j + 1) * NH],
                                pt0[:, sl],
                                start=(a == 0),
                                stop=(a == 7),
                            )
                            nc.tensor.matmul(
                                o_ps1[:, sl],
                                vt_sb[a][:, (2 * j + 1) * NH : (2 * j + 2) * NH],
                                pt1[:, sl],
                                start=(a == 0),
                                stop=(a == 7),
                            )

                # normalize: row 64 of o_ps holds the softmax denominator.
                # Evacuate PSUM fast on DVE (dens for both heads into one
                # [2,N] tile + unnormalized o into SBUF), then the slow
                # 1/den broadcast (DRAM bounce, stride-0 read) and the
                # multiply run on Pool, off the DVE/PSUM critical path.
                o_sb = []
                if merged_evac:
                    for slot, o_ps in ((0, o_ps0), (1, o_ps1)):
                        h = 2 * j + slot
                        t = norm_pool.tile(
                            [NH, N], F32, tag=f"osb_raw{slot}", name=f"o_raw{h}"
                        )
                        nc.vector.tensor_copy(t[:], o_ps[:])
                        o_sb.append(t)
                    r2 = norm_pool.tile([1, 2 * N], F32, tag="r2", name=f"r2_{j}")
                    for slot in (0, 1):
                        nc.vector.reciprocal_approx_fast(
                            r2[:, slot * N : (slot + 1) * N],
                            o_sb[slot][D : D + 1, :],
                        )
                else:
                    den2 = norm_pool.tile(
                        [1, 2 * N], F32, tag="den2", name=f"den2_{j}"
                    )
                    for slot, o_ps in ((0, o_ps0), (1, o_ps1)):
                        h = 2 * j + slot
                        nc.vector.tensor_copy(
                            den2[:, slot * N : (slot + 1) * N], o_ps[D : D + 1, :]
                        )
                        t = norm_pool.tile(
                            [D, N], F32, tag=f"osb_raw{slot}", name=f"o_raw{h}"
                        )
                        nc.vector.tensor_copy(t[:], o_ps[0:D, :])
                        o_sb.append(t)
                    r2 = norm_pool.tile([1, 2 * N], F32, tag="r2", name=f"r2_{j}")
                    nc.vector.reciprocal_approx_fast(r2[:], den2[:])
                den_dram = dram_pool.tile([1, 2 * N], F32, tag="den", name=f"den{j}")
                eng = (nc.sync, nc.scalar)[j % 2]
                eng.dma_start(den_dram[:], r2[:])
                for slot in (0, 1):
                    r_bc = norm_pool.tile(
                        [D, N], F32, tag=f"rbc{slot}", name=f"rbc{2 * j + slot}"
                    )
                    row = den_dram[:, slot * N : (slot + 1) * N]
                    bc_src = bass.AP(
                        tensor=row.tensor,
                        offset=row.offset,
                        ap=[[0, D]] + list(row.ap[1:]),
                    )
                    bc_eng = (nc.sync, nc.scalar)[slot] if split_bc else eng
                    bc_eng.dma_start(r_bc[:], bc_src)
                    base = slot * D
                    nc.gpsimd.tensor_mul(
                        osb[j][base : base + D, :], o_sb[slot][0:D, :], r_bc[:]
                    )

            def emit_compute():
                emit_B(0)
                emit_B(4)
                for n in range(8):
                    emit_C(n)
                for j in range(4):
                    if j < 3:

                        def mid(a, jn=j + 1):
                            if stagger_b:
                                if a == 3:
                                    emit_B(jn)
                                elif a == 5:
                                    emit_B(jn + 4)
                            elif a == 4:
                                emit_B(jn)
                                emit_B(jn + 4)

                        emit_pair(j, mid_work=mid)
                    else:
                        emit_pair(j)

                # ---- phase E: output projection (bias + residual on host) ---
                tags_e = ("s0", "s1", "o0", "o1")
                if e_m_outer:
                    for m in range(4):
                        psum = ps.tile([P, N], F32, tag=tags_e[m], name=f"psE{m}")
                        for e in range(4):
                            for ih in range(2):
                                sl = slice(ih * 512, (ih + 1) * 512)
                                nc.tensor.matmul(
                                    psum[:, sl],
                                    woutT[e][:, m * P : (m + 1) * P],
                                    osb[e][:, sl],
                                    start=(e == 0),
                                    stop=(e == 3),
                                )
                        out_sb = misc_pool.tile(
                            [P, N], F16, tag="outsb", name=f"out_sb{m}"
                        )
                        nc.vector.tensor_copy(out_sb[:], psum[:])
                        for ih in range(2):
                            sl = slice(ih * 512, (ih + 1) * 512)
                            eng = (nc.sync, nc.scalar, nc.scalar, nc.sync)[m]
                            eng.dma_start(
                                out_d[m * P : (m + 1) * P, sl], out_sb[:, sl]
                            )
                else:
                    psums = [
                        ps.tile([P, N], F32, tag=t_, name=f"psE{m}")
                        for m, t_ in enumerate(tags_e)
                    ]
                    for e in range(4):
                        for m in range(4):
                            for ih in range(2):
                                sl = slice(ih * 512, (ih + 1) * 512)
                                nc.tensor.matmul(
                                    psums[m][:, sl],
                                    woutT[e][:, m * P : (m + 1) * P],
                                    osb[e][:, sl],
                                    start=(e == 0),
                                    stop=(e == 3),
                                )
                    for m in range(4):
                        out_sb = misc_pool.tile(
                            [P, N], F16, tag="outsb", name=f"out_sb{m}"
                        )
                        nc.vector.tensor_copy(out_sb[:], psums[m][:])
                        for ih in range(2):
                            sl = slice(ih * 512, (ih + 1) * 512)
                            eng = (nc.sync, nc.scalar, nc.scalar, nc.sync)[m]
                            eng.dma_start(
                                out_d[m * P : (m + 1) * P, sl], out_sb[:, sl]
                            )

            with rep_inner:
                emit_compute()

    nc.compile()
    return nc


_CACHE = {}


def _get_nc(reps=1):
    key = reps
    if key not in _CACHE:
        _CACHE[key] = _build(reps=reps)
    return _CACHE[key]


def _in_maps(x, w_in, w_out, b_out):
    B = x.shape[0]
    xf = np.ascontiguousarray(x.reshape(B, C, N), dtype=np.float16)
    wqkT = np.ascontiguousarray(w_in[: 2 * E].T, dtype=np.float16)
    wvT = np.ascontiguousarray(w_in[2 * E :].T, dtype=np.float16)
    woutT = np.ascontiguousarray(w_out.T, dtype=np.float32)
    return [
        {"x": xf[b], "wqkT": wqkT, "wvT": wvT, "woutT": woutT}
        for b in range(B)
    ]


def kernel(x, w_in, w_out, b_out, heads):
    x = np.asarray(x)
    w_in = np.asarray(w_in)
    w_out = np.asarray(w_out)
    b_out = np.asarray(b_out)
    B = x.shape[0]
    assert int(heads) == HEADS, f"kernel compiled for heads=8, got {heads}"
    assert x.shape == (B, C, 32, 32) and B == N_CORES

    in_maps = _in_maps(x, w_in, w_out, b_out)
    nc = _get_nc()
    res = run_bass_kernel_spmd(nc, in_maps, core_ids=list(range(N_CORES)))
    out = np.stack([r["out"].astype(np.float32) for r in res.results])
    out = out.reshape(B, C, 32, 32) + x.astype(np.float32)
    out += b_out.astype(np.float32).reshape(1, C, 1, 1)
    return out.astype(x.dtype, copy=False)
